# revision 2
# baseline (speedup 1.0000x reference)
"""2-layer GAT forward on 8 Trainium2 NeuronCores.

Target-node sharding: nodes are degree-sorted and dealt round-robin to 8
cores in groups of 128 sharing a padded degree D_g.

Layer 1 does NOT gather: the host pre-expands source features per edge slot
into a per-core fp16 tensor x_edgesT [128 feats, total_slots] (pure data
layout), streamed sequentially while the tensor engine computes per-edge
projection + source score (one matmul per 128 edges, weights+a_src fused).
Pad slots hold a least-squares vector v with v@a_src = -150, so masking is
free (their exp underflows to 0). Softmax/aggregation run as contiguous
tree-folds split across vector+gpsimd; ELU mostly on the scalar engine.

Layer 2 tables [N, 8] are exchanged with segment-pipelined AllGathers
(overlapped with the tail of layer 1), expanded to 256B rows, and gathered
per edge with dma_gather across 4 SWDGE queues (descriptor-generation
pipelined against SDMA drain, which is the hard floor at ~4.5 ns/edge).
"""

import math
import numpy as np
import ml_dtypes

import concourse.bass as bass
import concourse.mybir as mybir
from concourse import bacc
from concourse.tile import TileContext
from concourse.bass_utils import run_bass_kernel_spmd
from concourse.masks import make_identity

FP16 = np.float16

NC = 8
P = 128
FIN = 128   # layer-1 input features
HF = 64     # H*F layer 1
H1 = 8
F1 = 8
C2 = 7      # layer-2 out features
R1 = 72     # per-edge row elems (64 feats + 8 s_src)
R2 = 64     # tab2 row elems (f32) = 256B
MASKVAL = -150.0

_CACHE = {}


# --------------------------------------------------------------------------
# device kernel builder
# --------------------------------------------------------------------------

def _build(npad, nslice, g_cnt, dg, total_slots):
    DT = mybir.dt
    fp32 = DT.float32
    fp16 = DT.float16
    base = 32768 if npad > 32768 else 0
    nc = bacc.Bacc("TRN2", target_bir_lowering=False, debug=False,
                   num_devices=NC, num_swdge_queues=4)

    xeT = nc.dram_tensor("xeT", [P, total_slots], fp16, kind="ExternalInput")
    xownT = nc.dram_tensor("xownT", [P, nslice], fp32, kind="ExternalInput")
    idx = nc.dram_tensor("idx", [P, total_slots // 16], DT.int16, kind="ExternalInput")
    srcmaskown = nc.dram_tensor("srcmaskown", [nslice], fp32, kind="ExternalInput")
    wcat1h = nc.dram_tensor("wcat1h", [P, R1], fp16, kind="ExternalInput")
    watrg = nc.dram_tensor("watrg", [P, 8], fp32, kind="ExternalInput")
    w2cat = nc.dram_tensor("w2cat", [HF, 16], fp32, kind="ExternalInput")
    b1d = nc.dram_tensor("b1d", [HF], fp32, kind="ExternalInput")
    b2d = nc.dram_tensor("b2d", [C2], fp32, kind="ExternalInput")
    out = nc.dram_tensor("out", [nslice, C2], fp32, kind="ExternalOutput")

    fracs = [0.30, 0.55, 0.75, 0.90, 1.0]
    NS = len(fracs)
    seg_hi = []
    for i, f in enumerate(fracs):
        h = max(int(round(f * g_cnt)), (seg_hi[i - 1] + 1) if i else 1)
        seg_hi.append(min(h, g_cnt))
    seg_hi[-1] = g_cnt
    seg_lo = [0] + seg_hi[:-1]
    tab2in = nc.dram_tensor("tab2in", [nslice, 8], fp32)
    tab2cS = [nc.dram_tensor(f"tab2c{i}", [NC * (seg_hi[i] - seg_lo[i]) * P, 8],
                             fp32, addr_space="Shared") for i in range(NS)]
    tab2f = nc.dram_tensor("tab2f", [npad, R2], fp32)

    offs = np.concatenate([[0], np.cumsum([P * d for d in dg])]).astype(int)

    with TileContext(nc) as tc:
        with (
            tc.tile_pool(name="persist", bufs=1) as pp,
            tc.tile_pool(name="pXe", bufs=2) as pxe,
            tc.tile_pool(name="pPs", bufs=3, space="PSUM") as pps,
            tc.tile_pool(name="pG1", bufs=2) as pg1,
            tc.tile_pool(name="pSc", bufs=2) as psc,
            tc.tile_pool(name="pMsg", bufs=1) as pmsg,
            tc.tile_pool(name="pSm", bufs=3) as psm,
            tc.tile_pool(name="pD_ps", bufs=3, space="PSUM") as pd_ps,
            tc.tile_pool(name="pIdx", bufs=6) as pidx,
            tc.tile_pool(name="pE_g", bufs=5) as pe_g,
            tc.tile_pool(name="pM2", bufs=1) as pm2,
            tc.tile_pool(name="pE_sm", bufs=3) as pe_sm,
        ):
            # ---- persistent small tiles ----
            wcat1_sb = pp.tile([P, R1], fp16, tag="wcat1")
            nc.sync.dma_start(out=wcat1_sb[:], in_=wcat1h[:])
            watrg_sb = pp.tile([P, 8], fp32, tag="watrg")
            nc.sync.dma_start(out=watrg_sb[:], in_=watrg[:])
            w2cat_sb = pp.tile([HF, 16], fp32, tag="w2cat")
            nc.sync.dma_start(out=w2cat_sb[:], in_=w2cat[:])
            srcmaskown_sb = pp.tile([P, g_cnt], fp32, tag="srcmaskown")
            nc.sync.dma_start(out=srcmaskown_sb[:], in_=srcmaskown.ap().rearrange("(g p) -> p g", p=P))
            ones_sb = pp.tile([1, P], fp32, tag="ones")
            nc.vector.memset(ones_sb[:], 1.0)
            b1_sb = pp.tile([1, HF], fp32, tag="b1sb")
            nc.sync.dma_start(out=b1_sb[:], in_=b1d.ap().rearrange("(o c) -> o c", o=1))
            b2m_sb = pp.tile([1, 8], fp32, tag="b2msb")
            nc.vector.memset(b2m_sb[:], 0.0)
            nc.sync.dma_start(out=b2m_sb[:, 0:C2], in_=b2d.ap().rearrange("(o c) -> o c", o=1))
            ident_sb = pp.tile([P, P], fp32, tag="ident")
            make_identity(nc, ident_sb[:])

            b1bc_ps = pd_ps.tile([P, HF], fp32, tag="dps")
            nc.tensor.matmul(out=b1bc_ps[:], lhsT=ones_sb[:], rhs=b1_sb[:], start=True, stop=True)
            b1bc_sb = pp.tile([P, HF], fp32, tag="b1bc")
            nc.vector.tensor_copy(out=b1bc_sb[:], in_=b1bc_ps[:])
            b2bc_ps = pd_ps.tile([P, 8], fp32, tag="dps")
            nc.tensor.matmul(out=b2bc_ps[:], lhsT=ones_sb[:], rhs=b2m_sb[:], start=True, stop=True)
            b2bc_sb = pp.tile([P, 8], fp32, tag="b2bc")
            nc.vector.tensor_copy(out=b2bc_sb[:], in_=b2bc_ps[:])

            tab2slice_sb = pp.tile([P, g_cnt * 8], fp32, tag="tab2slice")
            strg2_sb = pp.tile([P, g_cnt], fp32, tag="strg2")
            strgown_sb = pp.tile([P, g_cnt * 8], fp32, tag="strgown")

            # own-node s_trg1 via small matmuls on xownT
            for g in range(g_cnt):
                xog = pxe.tile([P, P], fp32, tag="xog")
                nc.sync.dma_start(out=xog[:], in_=xownT[:, g * P:(g + 1) * P])
                pso = pd_ps.tile([P, 8], fp32, tag="dps")
                nc.tensor.matmul(out=pso[:], lhsT=xog[:],
                                 rhs=watrg_sb[:], start=True, stop=True)
                nc.vector.tensor_copy(out=strgown_sb[:, g * 8:(g + 1) * 8], in_=pso[:])

            # ---- phases B and D, per group ----
            for g in range(g_cnt):
                D = dg[g]
                L = P * D
                xe = pxe.tile([P, L], fp16, tag="xe")
                nc.sync.dma_start(out=xe[:], in_=xeT[:, offs[g]:offs[g + 1]])
                g1 = pg1.tile([P, D * R1], fp32, tag="g1")
                CH = 7
                for c0 in range(0, D, CH):
                    cw = min(CH, D - c0)
                    ps = pps.tile([P, CH * R1], fp32, tag="projps")
                    for j in range(cw):
                        nc.tensor.matmul(out=ps[:, j * R1:(j + 1) * R1],
                                         lhsT=xe[:, (c0 + j) * P:(c0 + j + 1) * P],
                                         rhs=wcat1_sb[:], start=True, stop=True)
                    if (c0 // CH) % 2 == 0:
                        nc.scalar.copy(out=g1[:, c0 * R1:(c0 + cw) * R1],
                                       in_=ps[:, 0:cw * R1])
                    else:
                        nc.vector.tensor_copy(out=g1[:, c0 * R1:(c0 + cw) * R1],
                                              in_=ps[:, 0:cw * R1])
                g1v = g1[:].rearrange("p (d c) -> p d c", c=R1)

                sc = psc.tile([P, D * 8], fp32, tag="scores")
                scv = sc[:].rearrange("p (d h) -> p d h", h=H1)
                strg_g = strgown_sb[:, g * 8:(g + 1) * 8]
                nc.vector.tensor_add(
                    out=scv, in0=g1v[:, :, HF:R1],
                    in1=strg_g.rearrange("p (d h) -> p d h", d=1).to_broadcast([P, D, H1]))
                nc.vector.scalar_tensor_tensor(
                    out=sc[:], in0=sc[:], scalar=0.2, in1=sc[:],
                    op0=mybir.AluOpType.mult, op1=mybir.AluOpType.max)
                nc.scalar.activation(out=sc[:], in_=sc[:],
                                     func=mybir.ActivationFunctionType.Exp)

                # messages: front d-half on vector, back d-half on gpsimd
                msg = pmsg.tile([P, D * HF], fp32, tag="msg")
                DV = (D + 2) // 3
                mv = msg[:, 0:DV * HF]
                mg = msg[:, DV * HF:D * HF]
                nc.vector.tensor_mul(
                    out=mv.rearrange("p (d h f) -> p d h f", h=H1, f=F1),
                    in0=g1v[:, 0:DV, 0:HF].rearrange("p d (h f) -> p d h f", f=F1),
                    in1=sc[:, 0:DV * 8].rearrange("p (d h f) -> p d h f", h=H1, f=1
                                                  ).to_broadcast([P, DV, H1, F1]))
                if D > DV:
                    nc.gpsimd.tensor_mul(
                        out=mg.rearrange("p (d h f) -> p d h f", h=H1, f=F1),
                        in0=g1v[:, DV:D, 0:HF].rearrange("p d (h f) -> p d h f", f=F1),
                        in1=sc[:, DV * 8:D * 8].rearrange(
                            "p (d h f) -> p d h f", h=H1, f=1
                        ).to_broadcast([P, D - DV, H1, F1]))
                ssum = psm.tile([P, 8], fp32, tag="ssum")
                nc.vector.tensor_reduce(
                    out=ssum[:], in_=sc[:].rearrange("p (d h) -> p h d", h=H1),
                    axis=mybir.AxisListType.X, op=mybir.AluOpType.add)
                # contiguous tree-folds over the degree axis
                curv = DV
                while curv > 1:
                    half = curv // 2
                    lo = curv - half
                    nc.vector.tensor_add(out=msg[:, 0:half * HF],
                                         in0=msg[:, 0:half * HF],
                                         in1=msg[:, lo * HF:curv * HF])
                    curv = lo
                curg = D - DV
                while curg > 1:
                    half = curg // 2
                    lo = curg - half
                    nc.gpsimd.tensor_add(
                        out=msg[:, DV * HF:(DV + half) * HF],
                        in0=msg[:, DV * HF:(DV + half) * HF],
                        in1=msg[:, (DV + lo) * HF:(DV + curg) * HF])
                    curg = lo
                rinv = psm.tile([P, 8], fp32, tag="rinv")
                nc.vector.reciprocal(out=rinv[:], in_=ssum[:])
                if D > DV:
                    nc.vector.tensor_add(out=msg[:, 0:HF], in0=msg[:, 0:HF],
                                         in1=msg[:, DV * HF:(DV + 1) * HF])

                out1 = psm.tile([P, HF], fp32, tag="out1")
                nc.gpsimd.tensor_mul(
                    out=out1[:].rearrange("p (h f) -> p h f", h=H1),
                    in0=msg[:, 0:HF].rearrange("p (h f) -> p h f", h=H1),
                    in1=rinv[:].rearrange("p (h f) -> p h f", f=1
                                          ).to_broadcast([P, H1, F1]))
                nc.gpsimd.tensor_add(out=out1[:], in0=out1[:], in1=b1bc_sb[:])

                # ELU on scalar: relu(-z) -> exp(-.) ; relu(z) ; add ; -1
                uu = psm.tile([P, HF], fp32, tag="uu")
                nc.scalar.activation(out=uu[:], in_=out1[:],
                                     func=mybir.ActivationFunctionType.Relu,
                                     scale=-1.0)
                nc.scalar.activation(out=uu[:], in_=uu[:],
                                     func=mybir.ActivationFunctionType.Exp,
                                     scale=-1.0)
                nc.scalar.activation(out=out1[:], in_=out1[:],
                                     func=mybir.ActivationFunctionType.Relu)
                hh = psm.tile([P, HF], fp32, tag="hh")
                nc.gpsimd.tensor_add(out=hh[:], in0=uu[:], in1=out1[:])
                nc.scalar.activation(out=hh[:], in_=hh[:],
                                     func=mybir.ActivationFunctionType.Copy, bias=-1.0)

                # ---- phase D ----
                psT = pd_ps.tile([HF, P], fp32, tag="dps")
                nc.tensor.transpose(out=psT[:], in_=hh[:], identity=ident_sb[:])
                hT = psm.tile([HF, P], fp32, tag="hT")
                nc.vector.tensor_copy(out=hT[:], in_=psT[:])
                ps2 = pd_ps.tile([P, 9], fp32, tag="dps")
                nc.tensor.matmul(out=ps2[:], lhsT=hT[:], rhs=w2cat_sb[:, 0:9],
                                 start=True, stop=True)
                t2s = tab2slice_sb[:, g * 8:(g + 1) * 8]
                nc.vector.tensor_add(out=t2s, in0=ps2[:, 0:8], in1=b2bc_sb[:])
                nc.vector.tensor_add(
                    out=tab2slice_sb[:, g * 8 + 7:g * 8 + 8],
                    in0=tab2slice_sb[:, g * 8 + 7:g * 8 + 8],
                    in1=srcmaskown_sb[:, g:g + 1])
                nc.vector.tensor_copy(out=strg2_sb[:, g:g + 1], in_=ps2[:, 8:9])

                # ---- phase C, segment exchange: overlap with remaining groups
                for i in range(NS):
                    if g == seg_hi[i] - 1:
                        lo_r, hi_r = seg_lo[i] * P, seg_hi[i] * P
                        nr = hi_r - lo_r
                        nc.sync.dma_start(
                            out=tab2in.ap()[lo_r:hi_r, :].rearrange("(k p) c -> p k c", p=P),
                            in_=tab2slice_sb[:, seg_lo[i] * 8:seg_hi[i] * 8]
                            .rearrange("p (k c) -> p k c", c=8))
                        nc.gpsimd.collective_compute(
                            "AllGather",
                            mybir.AluOpType.bypass,
                            ins=[tab2in.ap()[lo_r:hi_r, :]],
                            outs=[tab2cS[i][:]],
                            replica_groups=[list(range(NC))],
                        )
                        nc.sync.dma_start(
                            out=tab2f.ap().rearrange("(c l) k -> c l k", c=NC)[:, lo_r:hi_r, 0:8],
                            in_=tab2cS[i].ap().rearrange("(c l) k -> c l k", l=nr))

            # ---- phase E: layer 2 per group ----
            for g in range(g_cnt):
                D = dg[g]
                L = P * D
                idxg = pidx.tile([P, (offs[g + 1] - offs[g]) // 16], DT.int16, tag="idxg")
                nc.sync.dma_start(out=idxg[:], in_=idx[:, offs[g] // 16:offs[g + 1] // 16])
                g2 = pe_g.tile([P, D * R2], fp32, tag="g2")
                nc.gpsimd.dma_gather(
                    out_ap=g2[:].rearrange("p (d c) -> p d c", c=R2),
                    in_ap=tab2f[base:, :],
                    idxs_ap=idxg[:],
                    num_idxs=L, num_idxs_reg=L, elem_size=R2,
                    single_packet=False, queue_num=g % 4)
                g2v = g2[:].rearrange("p (d c) -> p d c", c=R2)

                sc2 = pe_sm.tile([P, D], fp32, tag="sc2")
                nc.vector.tensor_scalar_add(
                    out=sc2[:],
                    in0=g2v[:, :, 7:8].rearrange("p d c -> p (d c)"),
                    scalar1=strg2_sb[:, g:g + 1])
                nc.vector.scalar_tensor_tensor(
                    out=sc2[:], in0=sc2[:], scalar=0.2, in1=sc2[:],
                    op0=mybir.AluOpType.mult, op1=mybir.AluOpType.max)
                ssum2 = pe_sm.tile([P, 1], fp32, tag="ssum2")
                nc.scalar.activation(out=sc2[:], in_=sc2[:],
                                     func=mybir.ActivationFunctionType.Exp,
                                     accum_out=ssum2[:])
                rinv2 = pe_sm.tile([P, 1], fp32, tag="rinv2")
                nc.vector.reciprocal(out=rinv2[:], in_=ssum2[:])

                m2 = pm2.tile([P, D * 8], fp32, tag="m2")
                nc.vector.tensor_mul(
                    out=m2[:].rearrange("p (d c) -> p d c", c=8),
                    in0=g2v[:, :, 0:8],
                    in1=sc2[:].rearrange("p (d c) -> p d c", c=1).to_broadcast([P, D, 8]))
                o2 = pe_sm.tile([P, 8], fp32, tag="o2")
                nc.vector.tensor_reduce(
                    out=o2[:], in_=m2[:].rearrange("p (d c) -> p c d", c=8),
                    axis=mybir.AxisListType.X, op=mybir.AluOpType.add)
                nc.vector.tensor_scalar_mul(out=o2[:], in0=o2[:], scalar1=rinv2[:])

                negmax = pe_sm.tile([P, 1], fp32, tag="negmax")
                nc.vector.tensor_reduce(
                    out=negmax[:], in_=o2[:, 0:C2], axis=mybir.AxisListType.X,
                    op=mybir.AluOpType.max, negate=True)
                sum7 = pe_sm.tile([P, 1], fp32, tag="sum7")
                e7 = pe_sm.tile([P, C2], fp32, tag="e7")
                nc.scalar.activation(out=e7[:], in_=o2[:, 0:C2],
                                     func=mybir.ActivationFunctionType.Exp,
                                     bias=negmax[:], accum_out=sum7[:])
                r7 = pe_sm.tile([P, 1], fp32, tag="r7")
                nc.vector.reciprocal(out=r7[:], in_=sum7[:])
                res = pe_sm.tile([P, C2], fp32, tag="res")
                nc.vector.tensor_scalar_mul(out=res[:], in0=e7[:], scalar1=r7[:])
                nc.sync.dma_start(out=out[g * P:(g + 1) * P, :], in_=res[:])

    nc.compile()
    return nc


# --------------------------------------------------------------------------
# host side
# --------------------------------------------------------------------------

def _preprocess(x, edge_index):
    src = np.asarray(edge_index[0], np.int64)
    trg = np.asarray(edge_index[1], np.int64)
    n = x.shape[0]
    e = src.shape[0]

    deg = np.bincount(trg, minlength=n)
    order = np.argsort(-deg, kind="stable")          # rank -> node
    g_cnt = math.ceil(n / (P * NC))
    if g_cnt * P * NC == n:
        g_cnt += 1  # ensure pad rows exist (dummy index must be a pad row)
    npad = g_cnt * P * NC
    nslice = g_cnt * P

    ranks = np.empty(n, np.int64)
    ranks[order] = np.arange(n)
    core_of = ranks % NC
    grp_of = ranks // (P * NC)
    slot_of = (ranks // NC) % P
    perm = core_of * nslice + grp_of * P + slot_of   # node -> perm position

    # per-group padded degree, shared across cores; make sure the LAST list
    # slot of each group is padding (trailing negative-idx trim on HW)
    dg = []
    for g in range(g_cnt):
        w = order[P * NC * g: P * NC * (g + 1)]
        if len(w) == 0:
            dg.append(1)
            continue
        degs = deg[w]  # already descending
        dmax = max(int(degs.max()), 1)
        if len(degs) <= 1016 or int(degs[1016:].max()) == dmax:
            dmax += 1
        dg.append(dmax)
    offs = np.concatenate([[0], np.cumsum([P * d for d in dg])]).astype(np.int64)
    total_slots = int(offs[-1])

    dummy = npad - 1  # a pad position
    base = 32768 if npad > 32768 else 0

    tp = perm[trg]
    eorder = np.argsort(tp, kind="stable")
    tps = tp[eorder]
    counts = np.bincount(tps, minlength=npad)
    starts = np.concatenate([[0], np.cumsum(counts)[:-1]])
    d_of = np.arange(e) - starts[tps]

    c_of = tps // nslice
    r_local = tps % nslice
    g_of = r_local // P
    p_of = r_local % P
    pos = offs[g_of] + d_of * P + p_of               # k = d*128 + p within group

    idx_flat = np.full((NC, total_slots), dummy - base, np.int16)
    idx_flat[c_of, pos] = (perm[src[eorder]] - base).astype(np.int16)

    # per-core source perm-row per slot (npad = the v pad row)
    srcperm = np.full((NC, total_slots), npad, np.int64)
    srcperm[c_of, pos] = perm[src[eorder]]

    # wrap for dma_gather: element k -> (partition k%16, col k//16), per group,
    # then replicate the 16-partition block to all 128 partitions
    idx_wrapped = np.empty((NC, P, total_slots // 16), np.int16)
    for g in range(g_cnt):
        lo, hi = int(offs[g]), int(offs[g + 1])
        blk = idx_flat[:, lo:hi].reshape(NC, (hi - lo) // 16, 16)  # [c, col, p16]
        wr = np.swapaxes(blk, 1, 2)                                # [c, p16, col]
        idx_wrapped[:, :, lo // 16:hi // 16] = np.tile(wr, (1, 8, 1))

    pad_mask = np.ones(npad, bool)
    pad_mask[perm] = False

    return dict(n=n, e=e, npad=npad, nslice=nslice, g_cnt=g_cnt, dg=dg,
                total_slots=total_slots, perm=perm, idx_wrapped=idx_wrapped,
                srcperm=srcperm, pad_mask=pad_mask, base=base)


def _prepare(x, edge_index, W1, a_src1, a_trg1, b1, W2, a_src2, a_trg2, b2):
    x = np.asarray(x, np.float32)
    W1 = np.asarray(W1, np.float32)
    a_src1 = np.asarray(a_src1, np.float32)
    a_trg1 = np.asarray(a_trg1, np.float32)
    b1 = np.asarray(b1, np.float32)
    W2 = np.asarray(W2, np.float32)
    a_src2 = np.asarray(a_src2, np.float32)
    a_trg2 = np.asarray(a_trg2, np.float32)
    b2 = np.asarray(b2, np.float32)

    meta = _preprocess(x, edge_index)
    npad, nslice, g_cnt = meta["npad"], meta["nslice"], meta["g_cnt"]
    perm = meta["perm"]

    xp = np.zeros((npad, FIN), np.float32)
    xp[perm] = x

    srcmaskown = np.where(meta["pad_mask"][:nslice], np.float32(MASKVAL),
                          np.float32(0.0))  # same local pad pattern per core

    Wt = W1.T  # [128, 64], col = h*F + j
    w3 = W1.reshape(H1, F1, FIN)
    Asrc = np.einsum("hjf,hj->fh", w3, a_src1[0])
    Atrg = np.einsum("hjf,hj->fh", w3, a_trg1[0])
    wcat1h = np.ascontiguousarray(
        np.concatenate([Wt, Asrc], axis=1)).astype(FP16)   # [128, 72]

    # pad vector: v @ Asrc = MASKVAL*ones (min-norm; Asrc cols are linear
    # combos of W cols, so the W block cannot be zeroed as well — pad proj
    # features stay finite junk, killed by exp(score)->0)
    A = wcat1h[:, HF:R1].astype(np.float32)            # [128, 8]
    t = np.full(8, MASKVAL, np.float32)
    v = np.linalg.lstsq(A.T, t, rcond=None)[0]         # [128]
    resid = A.T @ v - t
    assert np.abs(resid).max() < 1.0, resid
    assert np.abs(v).max() < 6e4, np.abs(v).max()

    xp16 = xp.astype(FP16)
    xT16_ext = np.ascontiguousarray(
        np.concatenate([xp16, v.astype(FP16)[None]], axis=0).T)  # [128, npad+1]

    w2cat = np.zeros((HF, 16), np.float32)
    w2cat[:, 0:C2] = W2.T
    w2cat[:, C2] = W2.T @ a_src2[0, 0]
    w2cat[:, C2 + 1] = W2.T @ a_trg2[0, 0]

    key = (npad, g_cnt, tuple(meta["dg"]))
    if key not in _CACHE:
        _CACHE[key] = _build(npad, nslice, g_cnt, meta["dg"], meta["total_slots"])
    nc = _CACHE[key]

    in_maps = []
    for c in range(NC):
        xownT = np.ascontiguousarray(xp[c * nslice:(c + 1) * nslice].T)
        xeT = np.ascontiguousarray(xT16_ext[:, meta["srcperm"][c]])
        in_maps.append({
            "xeT": xeT,
            "xownT": xownT,
            "idx": np.ascontiguousarray(meta["idx_wrapped"][c]),
            "srcmaskown": srcmaskown,
            "wcat1h": wcat1h,
            "watrg": np.ascontiguousarray(Atrg),
            "w2cat": w2cat,
            "b1d": b1,
            "b2d": b2,
        })
    return nc, in_maps, meta


def kernel(x, edge_index, W1, a_src1, a_trg1, b1, W2, a_src2, a_trg2, b2):
    nc, in_maps, meta = _prepare(x, edge_index, W1, a_src1, a_trg1, b1,
                                 W2, a_src2, a_trg2, b2)
    res = run_bass_kernel_spmd(nc, in_maps, core_ids=list(range(NC)))
    full = np.concatenate([res.results[c]["out"] for c in range(NC)], axis=0)
    return full[meta["perm"]].astype(np.float32)


# revision 3
# speedup vs baseline: 1.0853x; 1.0853x over previous
"""2-layer GAT forward on 8 Trainium2 NeuronCores.

Target-node sharding: nodes are degree-sorted and dealt round-robin to 8
cores in groups of 128 sharing a padded degree D_g.

Layer 1 does NOT gather: the host pre-expands source features per edge slot
into a per-core fp16 tensor x_edgesT [128 feats, total_slots] (pure data
layout), streamed sequentially while the tensor engine computes per-edge
projection + source score (one matmul per 128 edges, weights+a_src fused).
Pad slots hold a least-squares vector v with v@a_src = -150, so masking is
free (their exp underflows to 0). Softmax/aggregation run as contiguous
tree-folds split across vector+gpsimd; ELU mostly on the scalar engine.

Layer 2 tables [N, 8] are exchanged with segment-pipelined AllGathers
(overlapped with the tail of layer 1), expanded to 256B rows, and gathered
per edge with dma_gather across 4 SWDGE queues (descriptor-generation
pipelined against SDMA drain, which is the hard floor at ~4.5 ns/edge).
"""

import math
import numpy as np
import ml_dtypes

import concourse.bass as bass
import concourse.mybir as mybir
from concourse import bacc
from concourse.tile import TileContext
from concourse.bass_utils import run_bass_kernel_spmd
from concourse.masks import make_identity

FP16 = np.float16

NC = 8
P = 128
FIN = 128   # layer-1 input features
HF = 64     # H*F layer 1
H1 = 8
F1 = 8
C2 = 7      # layer-2 out features
R1 = 72     # per-edge row elems (64 feats + 8 s_src)
R2 = 64     # tab2 row elems (f32) = 256B
MASKVAL = -150.0

_CACHE = {}


# --------------------------------------------------------------------------
# device kernel builder
# --------------------------------------------------------------------------

def _build(npad, nslice, g_cnt, dg, total_slots):
    DT = mybir.dt
    fp32 = DT.float32
    fp16 = DT.float16
    bf16 = DT.bfloat16
    base = 32768 if npad > 32768 else 0
    nc = bacc.Bacc("TRN2", target_bir_lowering=False, debug=False,
                   num_devices=NC, num_swdge_queues=4)

    xeT = nc.dram_tensor("xeT", [P, total_slots], fp16, kind="ExternalInput")
    xownT = nc.dram_tensor("xownT", [P, nslice], fp32, kind="ExternalInput")
    idx = nc.dram_tensor("idx", [P, total_slots // 16], DT.int16, kind="ExternalInput")
    srcmaskown = nc.dram_tensor("srcmaskown", [nslice], fp32, kind="ExternalInput")
    wcat1h = nc.dram_tensor("wcat1h", [P, R1], fp16, kind="ExternalInput")
    watrg = nc.dram_tensor("watrg", [P, 8], fp32, kind="ExternalInput")
    w2cat = nc.dram_tensor("w2cat", [HF, 16], fp32, kind="ExternalInput")
    b1d = nc.dram_tensor("b1d", [HF], fp32, kind="ExternalInput")
    b2d = nc.dram_tensor("b2d", [C2], fp32, kind="ExternalInput")
    out = nc.dram_tensor("out", [nslice, C2], fp32, kind="ExternalOutput")

    fracs = [0.25, 0.45, 0.62, 0.78, 0.91, 1.0]
    NS = len(fracs)
    seg_hi = []
    for i, f in enumerate(fracs):
        h = max(int(round(f * g_cnt)), (seg_hi[i - 1] + 1) if i else 1)
        seg_hi.append(min(h, g_cnt))
    seg_hi[-1] = g_cnt
    seg_lo = [0] + seg_hi[:-1]
    tab2in = nc.dram_tensor("tab2in", [nslice, 8], fp32)
    tab2cS = [nc.dram_tensor(f"tab2c{i}", [NC * (seg_hi[i] - seg_lo[i]) * P, 8],
                             fp32, addr_space="Shared") for i in range(NS)]
    tab2f = nc.dram_tensor("tab2f", [npad, R2], fp32)

    offs = np.concatenate([[0], np.cumsum([P * d for d in dg])]).astype(int)

    with TileContext(nc) as tc:
        with (
            tc.tile_pool(name="persist", bufs=1) as pp,
            tc.tile_pool(name="pXe", bufs=2) as pxe,
            tc.tile_pool(name="pPs", bufs=3, space="PSUM") as pps,
            tc.tile_pool(name="pG1", bufs=2) as pg1,
            tc.tile_pool(name="pSc", bufs=2) as psc,
            tc.tile_pool(name="pMsg", bufs=1) as pmsg,
            tc.tile_pool(name="pSm", bufs=3) as psm,
            tc.tile_pool(name="pD_ps", bufs=3, space="PSUM") as pd_ps,
            tc.tile_pool(name="pIdx", bufs=6) as pidx,
            tc.tile_pool(name="pE_g", bufs=7) as pe_g,
            tc.tile_pool(name="pM2", bufs=2) as pm2,
            tc.tile_pool(name="pE_sm", bufs=4) as pe_sm,
        ):
            # ---- persistent small tiles ----
            wcat1_sb = pp.tile([P, R1], fp16, tag="wcat1")
            nc.sync.dma_start(out=wcat1_sb[:], in_=wcat1h[:])
            watrg_sb = pp.tile([P, 8], fp32, tag="watrg")
            nc.sync.dma_start(out=watrg_sb[:], in_=watrg[:])
            w2cat_sb = pp.tile([HF, 16], fp32, tag="w2cat")
            nc.sync.dma_start(out=w2cat_sb[:], in_=w2cat[:])
            srcmaskown_sb = pp.tile([P, g_cnt], fp32, tag="srcmaskown")
            nc.sync.dma_start(out=srcmaskown_sb[:], in_=srcmaskown.ap().rearrange("(g p) -> p g", p=P))
            ones_sb = pp.tile([1, P], fp32, tag="ones")
            nc.vector.memset(ones_sb[:], 1.0)
            b1_sb = pp.tile([1, HF], fp32, tag="b1sb")
            nc.sync.dma_start(out=b1_sb[:], in_=b1d.ap().rearrange("(o c) -> o c", o=1))
            b2m_sb = pp.tile([1, 8], fp32, tag="b2msb")
            nc.vector.memset(b2m_sb[:], 0.0)
            nc.sync.dma_start(out=b2m_sb[:, 0:C2], in_=b2d.ap().rearrange("(o c) -> o c", o=1))
            ident_sb = pp.tile([P, P], fp32, tag="ident")
            make_identity(nc, ident_sb[:])

            b1bc_ps = pd_ps.tile([P, HF], fp32, tag="dps")
            nc.tensor.matmul(out=b1bc_ps[:], lhsT=ones_sb[:], rhs=b1_sb[:], start=True, stop=True)
            b1bc_sb = pp.tile([P, HF], fp32, tag="b1bc")
            nc.vector.tensor_copy(out=b1bc_sb[:], in_=b1bc_ps[:])
            b2bc_ps = pd_ps.tile([P, 8], fp32, tag="dps")
            nc.tensor.matmul(out=b2bc_ps[:], lhsT=ones_sb[:], rhs=b2m_sb[:], start=True, stop=True)
            b2bc_sb = pp.tile([P, 8], fp32, tag="b2bc")
            nc.vector.tensor_copy(out=b2bc_sb[:], in_=b2bc_ps[:])

            tab2slice_sb = pp.tile([P, g_cnt * 8], fp32, tag="tab2slice")
            strg2_sb = pp.tile([P, g_cnt], fp32, tag="strg2")
            strgown_sb = pp.tile([P, g_cnt * 8], fp32, tag="strgown")

            # own-node s_trg1 via small matmuls on xownT
            for g in range(g_cnt):
                xog = pxe.tile([P, P], fp32, tag="xog")
                nc.sync.dma_start(out=xog[:], in_=xownT[:, g * P:(g + 1) * P])
                pso = pd_ps.tile([P, 8], fp32, tag="dps")
                nc.tensor.matmul(out=pso[:], lhsT=xog[:],
                                 rhs=watrg_sb[:], start=True, stop=True)
                nc.vector.tensor_copy(out=strgown_sb[:, g * 8:(g + 1) * 8], in_=pso[:])

            # ---- phases B and D, per group ----
            for g in range(g_cnt):
                D = dg[g]
                L = P * D
                xe = pxe.tile([P, L], fp16, tag="xe")
                nc.sync.dma_start(out=xe[:], in_=xeT[:, offs[g]:offs[g + 1]])
                g1 = pg1.tile([P, D * R1], bf16, tag="g1")
                CH = 7
                for c0 in range(0, D, CH):
                    cw = min(CH, D - c0)
                    ps = pps.tile([P, CH * R1], fp32, tag="projps")
                    for j in range(cw):
                        nc.tensor.matmul(out=ps[:, j * R1:(j + 1) * R1],
                                         lhsT=xe[:, (c0 + j) * P:(c0 + j + 1) * P],
                                         rhs=wcat1_sb[:], start=True, stop=True)
                    if (c0 // CH) % 2 == 0:
                        nc.scalar.copy(out=g1[:, c0 * R1:(c0 + cw) * R1],
                                       in_=ps[:, 0:cw * R1])
                    else:
                        nc.vector.tensor_copy(out=g1[:, c0 * R1:(c0 + cw) * R1],
                                              in_=ps[:, 0:cw * R1])
                g1v = g1[:].rearrange("p (d c) -> p d c", c=R1)

                sc = psc.tile([P, D * 8], bf16, tag="scores")
                scv = sc[:].rearrange("p (d h) -> p d h", h=H1)
                strg_g = strgown_sb[:, g * 8:(g + 1) * 8]
                nc.vector.tensor_add(
                    out=scv, in0=g1v[:, :, HF:R1],
                    in1=strg_g.rearrange("p (d h) -> p d h", d=1).to_broadcast([P, D, H1]))
                nc.vector.scalar_tensor_tensor(
                    out=sc[:], in0=sc[:], scalar=0.2, in1=sc[:],
                    op0=mybir.AluOpType.mult, op1=mybir.AluOpType.max)
                nc.scalar.activation(out=sc[:], in_=sc[:],
                                     func=mybir.ActivationFunctionType.Exp)

                # messages: front d-half on vector, back d-half on gpsimd
                msg = pmsg.tile([P, D * HF], bf16, tag="msg")
                DV = (D + 2) // 3
                mv = msg[:, 0:DV * HF]
                mg = msg[:, DV * HF:D * HF]
                nc.vector.tensor_mul(
                    out=mv.rearrange("p (d h f) -> p d h f", h=H1, f=F1),
                    in0=g1v[:, 0:DV, 0:HF].rearrange("p d (h f) -> p d h f", f=F1),
                    in1=sc[:, 0:DV * 8].rearrange("p (d h f) -> p d h f", h=H1, f=1
                                                  ).to_broadcast([P, DV, H1, F1]))
                if D > DV:
                    nc.gpsimd.tensor_mul(
                        out=mg.rearrange("p (d h f) -> p d h f", h=H1, f=F1),
                        in0=g1v[:, DV:D, 0:HF].rearrange("p d (h f) -> p d h f", f=F1),
                        in1=sc[:, DV * 8:D * 8].rearrange(
                            "p (d h f) -> p d h f", h=H1, f=1
                        ).to_broadcast([P, D - DV, H1, F1]))
                ssum = psm.tile([P, 8], fp32, tag="ssum")
                nc.vector.tensor_reduce(
                    out=ssum[:], in_=sc[:].rearrange("p (d h) -> p h d", h=H1),
                    axis=mybir.AxisListType.X, op=mybir.AluOpType.add)
                # contiguous tree-folds over the degree axis
                curv = DV
                while curv > 1:
                    half = curv // 2
                    lo = curv - half
                    nc.vector.tensor_add(out=msg[:, 0:half * HF],
                                         in0=msg[:, 0:half * HF],
                                         in1=msg[:, lo * HF:curv * HF])
                    curv = lo
                curg = D - DV
                while curg > 1:
                    half = curg // 2
                    lo = curg - half
                    nc.gpsimd.tensor_add(
                        out=msg[:, DV * HF:(DV + half) * HF],
                        in0=msg[:, DV * HF:(DV + half) * HF],
                        in1=msg[:, (DV + lo) * HF:(DV + curg) * HF])
                    curg = lo
                rinv = psm.tile([P, 8], fp32, tag="rinv")
                nc.vector.reciprocal(out=rinv[:], in_=ssum[:])
                if D > DV:
                    nc.vector.tensor_add(out=msg[:, 0:HF], in0=msg[:, 0:HF],
                                         in1=msg[:, DV * HF:(DV + 1) * HF])

                out1 = psm.tile([P, HF], fp32, tag="out1")
                nc.gpsimd.tensor_mul(
                    out=out1[:].rearrange("p (h f) -> p h f", h=H1),
                    in0=msg[:, 0:HF].rearrange("p (h f) -> p h f", h=H1),
                    in1=rinv[:].rearrange("p (h f) -> p h f", f=1
                                          ).to_broadcast([P, H1, F1]))
                nc.gpsimd.tensor_add(out=out1[:], in0=out1[:], in1=b1bc_sb[:])

                # ELU on scalar: relu(-z) -> exp(-.) ; relu(z) ; add ; -1
                uu = psm.tile([P, HF], fp32, tag="uu")
                nc.scalar.activation(out=uu[:], in_=out1[:],
                                     func=mybir.ActivationFunctionType.Relu,
                                     scale=-1.0)
                nc.scalar.activation(out=uu[:], in_=uu[:],
                                     func=mybir.ActivationFunctionType.Exp,
                                     scale=-1.0)
                nc.scalar.activation(out=out1[:], in_=out1[:],
                                     func=mybir.ActivationFunctionType.Relu)
                hh = psm.tile([P, HF], fp32, tag="hh")
                nc.gpsimd.tensor_add(out=hh[:], in0=uu[:], in1=out1[:])
                nc.scalar.activation(out=hh[:], in_=hh[:],
                                     func=mybir.ActivationFunctionType.Copy, bias=-1.0)

                # ---- phase D ----
                psT = pd_ps.tile([HF, P], fp32, tag="dps")
                nc.tensor.transpose(out=psT[:], in_=hh[:], identity=ident_sb[:])
                hT = psm.tile([HF, P], fp32, tag="hT")
                nc.vector.tensor_copy(out=hT[:], in_=psT[:])
                ps2 = pd_ps.tile([P, 9], fp32, tag="dps")
                nc.tensor.matmul(out=ps2[:], lhsT=hT[:], rhs=w2cat_sb[:, 0:9],
                                 start=True, stop=True)
                t2s = tab2slice_sb[:, g * 8:(g + 1) * 8]
                nc.vector.tensor_add(out=t2s, in0=ps2[:, 0:8], in1=b2bc_sb[:])
                nc.vector.tensor_add(
                    out=tab2slice_sb[:, g * 8 + 7:g * 8 + 8],
                    in0=tab2slice_sb[:, g * 8 + 7:g * 8 + 8],
                    in1=srcmaskown_sb[:, g:g + 1])
                nc.vector.tensor_copy(out=strg2_sb[:, g:g + 1], in_=ps2[:, 8:9])

                # ---- phase C, segment exchange: overlap with remaining groups
                for i in range(NS):
                    if g == seg_hi[i] - 1:
                        lo_r, hi_r = seg_lo[i] * P, seg_hi[i] * P
                        nr = hi_r - lo_r
                        nc.sync.dma_start(
                            out=tab2in.ap()[lo_r:hi_r, :].rearrange("(k p) c -> p k c", p=P),
                            in_=tab2slice_sb[:, seg_lo[i] * 8:seg_hi[i] * 8]
                            .rearrange("p (k c) -> p k c", c=8))
                        nc.gpsimd.collective_compute(
                            "AllGather",
                            mybir.AluOpType.bypass,
                            ins=[tab2in.ap()[lo_r:hi_r, :]],
                            outs=[tab2cS[i][:]],
                            replica_groups=[list(range(NC))],
                        )
                        nc.sync.dma_start(
                            out=tab2f.ap().rearrange("(c l) k -> c l k", c=NC)[:, lo_r:hi_r, 0:8],
                            in_=tab2cS[i].ap().rearrange("(c l) k -> c l k", l=nr))

            # ---- phase E: layer 2 per group ----
            for g in range(g_cnt):
                D = dg[g]
                L = P * D
                idxg = pidx.tile([P, (offs[g + 1] - offs[g]) // 16], DT.int16, tag="idxg")
                nc.sync.dma_start(out=idxg[:], in_=idx[:, offs[g] // 16:offs[g + 1] // 16])
                g2 = pe_g.tile([P, D * R2], fp32, tag="g2")
                nc.gpsimd.dma_gather(
                    out_ap=g2[:].rearrange("p (d c) -> p d c", c=R2),
                    in_ap=tab2f[base:, :],
                    idxs_ap=idxg[:],
                    num_idxs=L, num_idxs_reg=L, elem_size=R2,
                    single_packet=False, queue_num=g % 4)
                g2v = g2[:].rearrange("p (d c) -> p d c", c=R2)

                sc2 = pe_sm.tile([P, D], bf16, tag="sc2")
                nc.vector.tensor_scalar_add(
                    out=sc2[:],
                    in0=g2v[:, :, 7:8].rearrange("p d c -> p (d c)"),
                    scalar1=strg2_sb[:, g:g + 1])
                nc.vector.scalar_tensor_tensor(
                    out=sc2[:], in0=sc2[:], scalar=0.2, in1=sc2[:],
                    op0=mybir.AluOpType.mult, op1=mybir.AluOpType.max)
                ssum2 = pe_sm.tile([P, 1], fp32, tag="ssum2")
                nc.scalar.activation(out=sc2[:], in_=sc2[:],
                                     func=mybir.ActivationFunctionType.Exp,
                                     accum_out=ssum2[:])
                rinv2 = pe_sm.tile([P, 1], fp32, tag="rinv2")
                nc.vector.reciprocal(out=rinv2[:], in_=ssum2[:])

                m2 = pm2.tile([P, D * 8], bf16, tag="m2")
                nc.vector.tensor_mul(
                    out=m2[:].rearrange("p (d c) -> p d c", c=8),
                    in0=g2v[:, :, 0:8],
                    in1=sc2[:].rearrange("p (d c) -> p d c", c=1).to_broadcast([P, D, 8]))
                o2 = pe_sm.tile([P, 8], fp32, tag="o2")
                nc.vector.tensor_reduce(
                    out=o2[:], in_=m2[:].rearrange("p (d c) -> p c d", c=8),
                    axis=mybir.AxisListType.X, op=mybir.AluOpType.add)
                nc.vector.tensor_scalar_mul(out=o2[:], in0=o2[:], scalar1=rinv2[:])

                negmax = pe_sm.tile([P, 1], fp32, tag="negmax")
                nc.vector.tensor_reduce(
                    out=negmax[:], in_=o2[:, 0:C2], axis=mybir.AxisListType.X,
                    op=mybir.AluOpType.max, negate=True)
                sum7 = pe_sm.tile([P, 1], fp32, tag="sum7")
                e7 = pe_sm.tile([P, C2], fp32, tag="e7")
                nc.scalar.activation(out=e7[:], in_=o2[:, 0:C2],
                                     func=mybir.ActivationFunctionType.Exp,
                                     bias=negmax[:], accum_out=sum7[:])
                r7 = pe_sm.tile([P, 1], fp32, tag="r7")
                nc.vector.reciprocal(out=r7[:], in_=sum7[:])
                res = pe_sm.tile([P, C2], fp32, tag="res")
                nc.vector.tensor_scalar_mul(out=res[:], in0=e7[:], scalar1=r7[:])
                nc.sync.dma_start(out=out[g * P:(g + 1) * P, :], in_=res[:])

    nc.compile()
    return nc


# --------------------------------------------------------------------------
# host side
# --------------------------------------------------------------------------

def _preprocess(x, edge_index):
    src = np.asarray(edge_index[0], np.int64)
    trg = np.asarray(edge_index[1], np.int64)
    n = x.shape[0]
    e = src.shape[0]

    deg = np.bincount(trg, minlength=n)
    order = np.argsort(-deg, kind="stable")          # rank -> node
    g_cnt = math.ceil(n / (P * NC))
    if g_cnt * P * NC == n:
        g_cnt += 1  # ensure pad rows exist (dummy index must be a pad row)
    npad = g_cnt * P * NC
    nslice = g_cnt * P

    ranks = np.empty(n, np.int64)
    ranks[order] = np.arange(n)
    core_of = ranks % NC
    grp_of = ranks // (P * NC)
    slot_of = (ranks // NC) % P
    perm = core_of * nslice + grp_of * P + slot_of   # node -> perm position

    # per-group padded degree, shared across cores; make sure the LAST list
    # slot of each group is padding (trailing negative-idx trim on HW)
    dg = []
    for g in range(g_cnt):
        w = order[P * NC * g: P * NC * (g + 1)]
        if len(w) == 0:
            dg.append(1)
            continue
        degs = deg[w]  # already descending
        dmax = max(int(degs.max()), 1)
        if len(degs) <= 1016 or int(degs[1016:].max()) == dmax:
            dmax += 1
        dg.append(dmax)
    offs = np.concatenate([[0], np.cumsum([P * d for d in dg])]).astype(np.int64)
    total_slots = int(offs[-1])

    dummy = npad - 1  # a pad position
    base = 32768 if npad > 32768 else 0

    tp = perm[trg]
    eorder = np.argsort(tp, kind="stable")
    tps = tp[eorder]
    counts = np.bincount(tps, minlength=npad)
    starts = np.concatenate([[0], np.cumsum(counts)[:-1]])
    d_of = np.arange(e) - starts[tps]

    c_of = tps // nslice
    r_local = tps % nslice
    g_of = r_local // P
    p_of = r_local % P
    pos = offs[g_of] + d_of * P + p_of               # k = d*128 + p within group

    idx_flat = np.full((NC, total_slots), dummy - base, np.int16)
    idx_flat[c_of, pos] = (perm[src[eorder]] - base).astype(np.int16)

    # per-core source perm-row per slot (npad = the v pad row)
    srcperm = np.full((NC, total_slots), npad, np.int64)
    srcperm[c_of, pos] = perm[src[eorder]]

    # wrap for dma_gather: element k -> (partition k%16, col k//16), per group,
    # then replicate the 16-partition block to all 128 partitions
    idx_wrapped = np.empty((NC, P, total_slots // 16), np.int16)
    for g in range(g_cnt):
        lo, hi = int(offs[g]), int(offs[g + 1])
        blk = idx_flat[:, lo:hi].reshape(NC, (hi - lo) // 16, 16)  # [c, col, p16]
        wr = np.swapaxes(blk, 1, 2)                                # [c, p16, col]
        idx_wrapped[:, :, lo // 16:hi // 16] = np.tile(wr, (1, 8, 1))

    pad_mask = np.ones(npad, bool)
    pad_mask[perm] = False

    return dict(n=n, e=e, npad=npad, nslice=nslice, g_cnt=g_cnt, dg=dg,
                total_slots=total_slots, perm=perm, idx_wrapped=idx_wrapped,
                srcperm=srcperm, pad_mask=pad_mask, base=base)


def _prepare(x, edge_index, W1, a_src1, a_trg1, b1, W2, a_src2, a_trg2, b2):
    x = np.asarray(x, np.float32)
    W1 = np.asarray(W1, np.float32)
    a_src1 = np.asarray(a_src1, np.float32)
    a_trg1 = np.asarray(a_trg1, np.float32)
    b1 = np.asarray(b1, np.float32)
    W2 = np.asarray(W2, np.float32)
    a_src2 = np.asarray(a_src2, np.float32)
    a_trg2 = np.asarray(a_trg2, np.float32)
    b2 = np.asarray(b2, np.float32)

    meta = _preprocess(x, edge_index)
    npad, nslice, g_cnt = meta["npad"], meta["nslice"], meta["g_cnt"]
    perm = meta["perm"]

    xp = np.zeros((npad, FIN), np.float32)
    xp[perm] = x

    srcmaskown = np.where(meta["pad_mask"][:nslice], np.float32(MASKVAL),
                          np.float32(0.0))  # same local pad pattern per core

    Wt = W1.T  # [128, 64], col = h*F + j
    w3 = W1.reshape(H1, F1, FIN)
    Asrc = np.einsum("hjf,hj->fh", w3, a_src1[0])
    Atrg = np.einsum("hjf,hj->fh", w3, a_trg1[0])
    wcat1h = np.ascontiguousarray(
        np.concatenate([Wt, Asrc], axis=1)).astype(FP16)   # [128, 72]

    # pad vector: v @ Asrc = MASKVAL*ones (min-norm; Asrc cols are linear
    # combos of W cols, so the W block cannot be zeroed as well — pad proj
    # features stay finite junk, killed by exp(score)->0)
    A = wcat1h[:, HF:R1].astype(np.float32)            # [128, 8]
    t = np.full(8, MASKVAL, np.float32)
    v = np.linalg.lstsq(A.T, t, rcond=None)[0]         # [128]
    resid = A.T @ v - t
    assert np.abs(resid).max() < 1.0, resid
    assert np.abs(v).max() < 6e4, np.abs(v).max()

    xp16 = xp.astype(FP16)
    xT16_ext = np.ascontiguousarray(
        np.concatenate([xp16, v.astype(FP16)[None]], axis=0).T)  # [128, npad+1]

    w2cat = np.zeros((HF, 16), np.float32)
    w2cat[:, 0:C2] = W2.T
    w2cat[:, C2] = W2.T @ a_src2[0, 0]
    w2cat[:, C2 + 1] = W2.T @ a_trg2[0, 0]

    key = (npad, g_cnt, tuple(meta["dg"]))
    if key not in _CACHE:
        _CACHE[key] = _build(npad, nslice, g_cnt, meta["dg"], meta["total_slots"])
    nc = _CACHE[key]

    in_maps = []
    for c in range(NC):
        xownT = np.ascontiguousarray(xp[c * nslice:(c + 1) * nslice].T)
        xeT = np.ascontiguousarray(xT16_ext[:, meta["srcperm"][c]])
        in_maps.append({
            "xeT": xeT,
            "xownT": xownT,
            "idx": np.ascontiguousarray(meta["idx_wrapped"][c]),
            "srcmaskown": srcmaskown,
            "wcat1h": wcat1h,
            "watrg": np.ascontiguousarray(Atrg),
            "w2cat": w2cat,
            "b1d": b1,
            "b2d": b2,
        })
    return nc, in_maps, meta


def kernel(x, edge_index, W1, a_src1, a_trg1, b1, W2, a_src2, a_trg2, b2):
    nc, in_maps, meta = _prepare(x, edge_index, W1, a_src1, a_trg1, b1,
                                 W2, a_src2, a_trg2, b2)
    res = run_bass_kernel_spmd(nc, in_maps, core_ids=list(range(NC)))
    full = np.concatenate([res.results[c]["out"] for c in range(NC)], axis=0)
    return full[meta["perm"]].astype(np.float32)
